# revision 18
# baseline (speedup 1.0000x reference)
"""Non-local block (NLB) Trainium2 kernel.

Data-parallel over batch: 8 samples -> 8 NeuronCores, one sample per core.
Per core (C=128 channels, n=4096 pixels, inter=64):

  scores_T[m, n] = x_m^T (B x_n + v)        B = phi_w^T theta_w, v = phi_w^T theta_b
      (the per-n constant term theta_b . (phi_w x_n + phi_b) is dropped --
       softmax over m is invariant to it)
  probs_T = exp(scores_T)                   (scores bounded ~ +-22, fp32-safe,
                                             so no max-subtraction pass)
  y_aug[o, n]  = sum_m g_aug[m, o] probs_T[m, n]   with g_aug[:, 64] == 1
      -> row 64 is the softmax row-sum; g_aug[:, 0:64] = x^T g_w^T
  out[c, n] = (out_w @ y_aug[0:64]) [c, n] / y_aug[64, n] + out_b_eff[c] + x[c, n]
      out_b_eff = out_w @ g_b + out_b       (softmax rows sum to 1 => g_b folds)

Layouts keep the softmax (m) axis on partitions so exp runs straight out of
PSUM on ScalarE while the PE does scores / y matmuls in fp32r.
"""

import sys

if "/root/.axon_site/_ro/trn_rl_repo" not in sys.path:
    sys.path.insert(0, "/root/.axon_site/_ro/trn_rl_repo")

import types

import ml_dtypes
import numpy as np

import concourse.bass as bass
import concourse.mybir as mybir
import concourse.tile as tile
from concourse import bacc
from concourse import bass_utils

# The image's antenv package lacks axon_hooks; shim it so trace=True works.
try:
    import antenv.axon_hooks  # noqa: F401
except ImportError:
    try:
        import trn_agent_boot.trn_boot as _tb

        _hook = _tb._ntff_profile_via_ctypes("/opt/axon/libaxon_pjrt.so")
        _m = types.ModuleType("antenv.axon_hooks")
        _m.get_axon_ntff_profile_hook = lambda: _hook
        sys.modules["antenv.axon_hooks"] = _m
    except Exception:
        pass

B, C, H, W = 8, 128, 64, 64
N = H * W          # 4096 pixels
INTER = C // 2     # 64
P = 128
NCH = 1024         # n-chunk width (exp batching; 2 PSUM banks)
NSUB = 512         # matmul moving-dim width (1 PSUM bank)
MBLK = N // P      # 32 m-blocks
F32 = mybir.dt.float32
F32R = mybir.dt.float32r
BF16 = mybir.dt.bfloat16

_cached = {}


def _r(ap):
    """View an fp32 AP as float32r for full-rate PE streaming."""
    return ap.bitcast(F32R)


def build_nc():
    nc = bacc.Bacc("TRN2", target_bir_lowering=False, debug=False, num_devices=B)

    x_d = nc.dram_tensor("x", [P, N], F32, kind="ExternalInput")
    xbf_d = nc.dram_tensor("x_bf", [P, N], mybir.dt.bfloat16, kind="ExternalInput")
    ulhs_d = nc.dram_tensor("u_lhsT", [P, P], F32, kind="ExternalInput")
    v_d = nc.dram_tensor("v", [P, 1], F32, kind="ExternalInput")
    gwt_d = nc.dram_tensor("g_wT", [P, INTER], F32, kind="ExternalInput")
    owt_d = nc.dram_tensor("out_wT", [INTER, P], F32, kind="ExternalInput")
    obe_d = nc.dram_tensor("out_b_eff", [P, 1], F32, kind="ExternalInput")
    out_d = nc.dram_tensor("out", [P, N], F32, kind="ExternalOutput")

    EXP = mybir.ActivationFunctionType.Exp
    MULT = mybir.AluOpType.mult
    ADD = mybir.AluOpType.add

    with tile.TileContext(nc) as tc:
        const = tc.alloc_tile_pool(name="const", bufs=1)
        big = tc.alloc_tile_pool(name="big", bufs=1)
        probs_p = tc.alloc_tile_pool(name="probs", bufs=6)
        ysb_p = tc.alloc_tile_pool(name="ysb", bufs=2)
        rs_p = tc.alloc_tile_pool(name="rs", bufs=2)
        inv_p = tc.alloc_tile_pool(name="inv", bufs=2)
        osb_p = tc.alloc_tile_pool(name="osb", bufs=3)
        ssb_p = tc.alloc_tile_pool(name="ssb", bufs=2)

        # PSUM budget (8 banks): scores 3x2 + y 1x2 = 8; aux work borrows
        # scores slots (same tag, slot sized to the 1024-wide scores tile)
        sc_ps = tc.alloc_tile_pool(name="sc_ps", bufs=3, space="PSUM")
        aux_ps = sc_ps
        y_ps = tc.alloc_tile_pool(name="y_ps", bufs=1, space="PSUM")

        # ---- warm up ACT exp table immediately (no data deps) ----
        ones_col = const.tile([P, 1], F32, tag='ones_col')
        nc.vector.memset(ones_col[:], 1.0)
        warm = const.tile([1, 1], F32, tag='warm')
        nc.scalar.activation(warm[:], ones_col[0:1, :], EXP)

        # ---- constants / weights ----
        ulhs = const.tile([P, P], F32, tag='ulhs')
        nc.sync.dma_start(ulhs[:], ulhs_d.ap())
        v_sb = const.tile([P, 1], F32, tag='v')
        nc.gpsimd.dma_start(v_sb[:], v_d.ap())
        gwt = const.tile([P, INTER], F32, tag='gwt')
        nc.gpsimd.dma_start(gwt[:], gwt_d.ap())
        owt = const.tile([INTER, P], F32, tag='owt')
        nc.gpsimd.dma_start(owt[:], owt_d.ap())
        obe = const.tile([P, 1], F32, tag='obe')
        nc.gpsimd.dma_start(obe[:], obe_d.ap())
        ulhs_r = const.tile([P, P], BF16, tag='ulhsr')
        nc.vector.tensor_copy(ulhs_r[:], ulhs[:])
        gwt_r = const.tile([P, INTER], BF16, tag='gwtr')
        nc.vector.tensor_copy(gwt_r[:], gwt[:])
        owt_r = const.tile([INTER, P], BF16, tag='owtr')
        nc.vector.tensor_copy(owt_r[:], owt[:])
        ones_row = const.tile([1, P], F32, tag='ones_row')
        nc.vector.memset(ones_row[:], 1.0)
        e_col = const.tile([P, 1], F32, tag='e_col')
        nc.vector.memset(e_col[:], float(np.exp(1.0)))

        x_sb = big.tile([P, N], F32, tag='x')
        x_r = big.tile([P, N], BF16, tag='xr')
        u_sb = big.tile([P, N], BF16, tag='u')
        xpb = big.tile([P, N], F32, tag='xpb')
        g_aug = big.tile([P, MBLK * (INTER + 1)], BF16, tag='gaug')

        # ---- prologue: bf16 x lands first in small pieces, f32 x later ----
        xbf_slices = [(0, 512), (512, 512), (1024, 1024), (2048, 1024), (3072, 1024)]
        for (o, w) in xbf_slices:
            nc.sync.dma_start(x_r[:, o:o + w], xbf_d.ap()[:, o:o + w])
        for c in range(N // NCH):
            sl = slice(c * NCH, (c + 1) * NCH)
            nc.sync.dma_start(x_sb[:, sl], x_d.ap()[:, sl])
        for (o, w) in xbf_slices:
            for h in range(w // NSUB):
                sl = slice(o + h * NSUB, o + (h + 1) * NSUB)
                u_pt = aux_ps.tile([P, NSUB], F32, tag="s_t", name="s_t")
                nc.tensor.matmul(u_pt[:], ulhs_r[:], x_r[:, sl],
                                 start=True, stop=True)
                nc.vector.tensor_scalar_add(u_sb[:, sl], u_pt[:], v_sb[:])
            for j in range(o // P, (o + w) // P):
                g_pt = aux_ps.tile([P, INTER], F32, tag="s_t", name="s_t")
                nc.tensor.matmul(g_pt[:], x_r[:, j * P:(j + 1) * P], gwt_r[:],
                                 start=True, stop=True)
                nc.vector.tensor_copy(g_aug[:, j * 65:j * 65 + INTER], g_pt[:])
        for c in range(N // NCH):
            csl = slice(c * NCH, (c + 1) * NCH)
            nc.vector.tensor_scalar_add(xpb[:, csl], x_sb[:, csl], obe[:])
        g_ones_view = g_aug.rearrange("p (j t) -> p j t", t=INTER + 1)[:, :, INTER:INTER + 1]
        nc.vector.tensor_copy(g_ones_view, ones_col[:].to_broadcast([P, MBLK, 1]))

        # ---- main loop over n-chunks ----
        NC_CHUNKS = N // NCH
        GP_BLOCKS = ()              # gpsimd pow-exp offload: measured far too slow
        pending_out = []

        def emit_out_phase(c, y_t):
            n0 = c * NCH
            last = (c == NC_CHUNKS - 1)
            y_sb = ysb_p.tile([INTER, NCH], BF16, name="y_sb")
            nc.vector.tensor_copy(y_sb[:], y_t[0:INTER, :])
            rs_sb = rs_p.tile([1, NCH], F32, name="rs_sb")
            if last:
                nc.scalar.copy(rs_sb[:], y_t[INTER:INTER + 1, :])
            else:
                nc.vector.tensor_copy(rs_sb[:], y_t[INTER:INTER + 1, :])
            inv_sb = inv_p.tile([P, NCH], F32, tag='inv', name="inv_sb")
            if last:
                for h in range(NCH // NSUB):
                    sl = slice(h * NSUB, (h + 1) * NSUB)
                    bc_t = aux_ps.tile([P, NSUB], F32, tag="s_t", name="bc_t")
                    nc.tensor.matmul(bc_t[:], ones_row[:],
                                     rs_sb[:, sl], start=True, stop=True)
                    nc.vector.reciprocal_approx_fast(out=inv_sb[:, sl], in_=bc_t[:])
            else:
                rs_bc = inv_p.tile([P, NCH], F32, tag='rsbc', name="rs_bc")
                nc.gpsimd.partition_broadcast(rs_bc[:], rs_sb[:])
                nc.vector.reciprocal_approx_fast(out=inv_sb[:], in_=rs_bc[:])
            for h in range(NCH // NSUB):
                sl = slice(h * NSUB, (h + 1) * NSUB)
                ot = aux_ps.tile([P, NSUB], F32, tag="s_t", name="ot")
                nc.tensor.matmul(ot[:], owt_r[:], y_sb[:, sl],
                                 start=True, stop=True)
                t_sb = osb_p.tile([P, NSUB], F32, name="t_sb")
                nc.vector.tensor_tensor(t_sb[:], ot[:], inv_sb[:, sl], op=MULT)
                nc.vector.tensor_tensor(
                    t_sb[:], t_sb[:],
                    xpb[:, n0 + h * NSUB:n0 + (h + 1) * NSUB], op=ADD)
                nc.sync.dma_start(out_d.ap()[:, n0 + h * NSUB:n0 + (h + 1) * NSUB],
                                  t_sb[:])

        for c in range(NC_CHUNKS):
            n0 = c * NCH
            gp_blocks = GP_BLOCKS if c < NC_CHUNKS - 1 else ()
            act_blocks = [j for j in range(MBLK) if j not in gp_blocks]
            y_t = y_ps.tile([INTER + 1, NCH], F32)

            def scores_mm(j, s_t):
                for h in range(NCH // NSUB):
                    nc.tensor.matmul(
                        s_t[:, h * NSUB:(h + 1) * NSUB],
                        x_r[:, j * P:(j + 1) * P],
                        u_sb[:, n0 + h * NSUB:n0 + (h + 1) * NSUB],
                        start=True, stop=True)

            def y_mm(j, p_t, start, stop):
                for h in range(NCH // NSUB):
                    nc.tensor.matmul(
                        y_t[:, h * NSUB:(h + 1) * NSUB],
                        g_aug[:, j * 65:(j + 1) * 65],
                        p_t[:, h * NSUB:(h + 1) * NSUB],
                        start=start and h < 2, stop=stop)

            # first block's scores+exp go ahead of the previous chunk's
            # out-phase so the ACT stream doesn't stall at the boundary
            j0 = act_blocks[0]
            s_t = sc_ps.tile([P, NCH], F32, name="s_t")
            scores_mm(j0, s_t)
            p0 = probs_p.tile([P, NCH], BF16, name="p_t")
            nc.scalar.activation(p0[:], s_t[:], EXP)
            if pending_out:
                emit_out_phase(*pending_out.pop(0))
            y_mm(j0, p0, start=True, stop=False)

            for k, j in enumerate(act_blocks[1:]):
                s_t = sc_ps.tile([P, NCH], F32, name="s_t")
                scores_mm(j, s_t)
                p_t = probs_p.tile([P, NCH], BF16, name="p_t")
                nc.scalar.activation(p_t[:], s_t[:], EXP)
                y_mm(j, p_t, start=False, stop=(k == len(act_blocks) - 2))

            pending_out.append((c, y_t))

        while pending_out:
            emit_out_phase(*pending_out.pop(0))

        for p in (y_ps, sc_ps, ssb_p,
                  osb_p, inv_p, rs_p, ysb_p, probs_p, big, const):
            p.release()

    nc.compile()
    return nc


def _prep_inputs(x, theta_w, theta_b, phi_w, phi_b, g_w, g_b, out_w, out_b):
    f = np.float32
    x = np.asarray(x, f)
    theta_w = np.asarray(theta_w, f)
    theta_b = np.asarray(theta_b, f)
    phi_w = np.asarray(phi_w, f)
    phi_b = np.asarray(phi_b, f)
    g_w = np.asarray(g_w, f)
    g_b = np.asarray(g_b, f)
    out_w = np.asarray(out_w, f)
    out_b = np.asarray(out_b, f)

    u_lhsT = np.ascontiguousarray(theta_w.T @ phi_w)          # [c2, c1] = B^T
    v = np.ascontiguousarray((phi_w.T @ theta_b)[:, None])    # [128, 1]
    g_wT = np.ascontiguousarray(g_w.T)                        # [128, 64]
    out_wT = np.ascontiguousarray(out_w.T)                    # [64, 128]
    out_b_eff = np.ascontiguousarray((out_w @ g_b + out_b)[:, None])

    in_maps = []
    for b in range(B):
        in_maps.append({
            "x": np.ascontiguousarray(x[b].reshape(P, N)),
            "x_bf": np.ascontiguousarray(
                x[b].reshape(P, N).astype(ml_dtypes.bfloat16)),
            "u_lhsT": u_lhsT,
            "v": v,
            "g_wT": g_wT,
            "out_wT": out_wT,
            "out_b_eff": out_b_eff,
        })
    return in_maps


def run_on_device(inputs, trace=False, trace_cores=None):
    if "nc" not in _cached:
        _cached["nc"] = build_nc()
    nc = _cached["nc"]
    in_maps = _prep_inputs(**inputs)
    res = bass_utils.run_bass_kernel_spmd(
        nc, in_maps, core_ids=list(range(B)), trace=trace,
        trace_cores=trace_cores)
    out = np.stack([res.results[b]["out"] for b in range(B)], axis=0)
    return out.reshape(B, C, H, W).astype(np.float32), res


def kernel(**inputs):
    out, _ = run_on_device(inputs, trace=False)
    return out


# revision 20
# speedup vs baseline: 1.0383x; 1.0383x over previous
"""Non-local block (NLB) Trainium2 kernel.

Data-parallel over batch: 8 samples -> 8 NeuronCores, one sample per core.
Per core (C=128 channels, n=4096 pixels, inter=64):

  scores_T[m, n] = x_m^T (B x_n + v)        B = phi_w^T theta_w, v = phi_w^T theta_b
      (the per-n constant term theta_b . (phi_w x_n + phi_b) is dropped --
       softmax over m is invariant to it)
  probs_T = exp(scores_T)                   (scores bounded ~ +-22, fp32-safe,
                                             so no max-subtraction pass)
  y_aug[o, n]  = sum_m g_aug[m, o] probs_T[m, n]   with g_aug[:, 64] == 1
      -> row 64 is the softmax row-sum; g_aug[:, 0:64] = x^T g_w^T
  out[c, n] = (out_w @ y_aug[0:64]) [c, n] / y_aug[64, n] + out_b_eff[c] + x[c, n]
      out_b_eff = out_w @ g_b + out_b       (softmax rows sum to 1 => g_b folds)

Layouts keep the softmax (m) axis on partitions so exp runs straight out of
PSUM on ScalarE while the PE does scores / y matmuls in fp32r.
"""

import sys

if "/root/.axon_site/_ro/trn_rl_repo" not in sys.path:
    sys.path.insert(0, "/root/.axon_site/_ro/trn_rl_repo")

import types

import ml_dtypes
import numpy as np

import concourse.bass as bass
import concourse.mybir as mybir
import concourse.tile as tile
from concourse import bacc
from concourse import bass_utils

# The image's antenv package lacks axon_hooks; shim it so trace=True works.
try:
    import antenv.axon_hooks  # noqa: F401
except ImportError:
    try:
        import trn_agent_boot.trn_boot as _tb

        _hook = _tb._ntff_profile_via_ctypes("/opt/axon/libaxon_pjrt.so")
        _m = types.ModuleType("antenv.axon_hooks")
        _m.get_axon_ntff_profile_hook = lambda: _hook
        sys.modules["antenv.axon_hooks"] = _m
    except Exception:
        pass

B, C, H, W = 8, 128, 64, 64
N = H * W          # 4096 pixels
INTER = C // 2     # 64
P = 128
NCH = 1024         # n-chunk width (exp batching; 2 PSUM banks)
NSUB = 512         # matmul moving-dim width (1 PSUM bank)
MBLK = N // P      # 32 m-blocks
F32 = mybir.dt.float32
F32R = mybir.dt.float32r
BF16 = mybir.dt.bfloat16

_cached = {}


def _r(ap):
    """View an fp32 AP as float32r for full-rate PE streaming."""
    return ap.bitcast(F32R)


def build_nc():
    nc = bacc.Bacc("TRN2", target_bir_lowering=False, debug=False, num_devices=B)

    x_d = nc.dram_tensor("x", [P, N], F32, kind="ExternalInput")
    xbf_d = nc.dram_tensor("x_bf", [P, N], mybir.dt.bfloat16, kind="ExternalInput")
    ulhs_d = nc.dram_tensor("u_lhsT", [P, P], F32, kind="ExternalInput")
    v_d = nc.dram_tensor("v", [P, 1], F32, kind="ExternalInput")
    gwt_d = nc.dram_tensor("g_wT", [P, INTER], F32, kind="ExternalInput")
    owt_d = nc.dram_tensor("out_wT", [INTER, P], F32, kind="ExternalInput")
    obe_d = nc.dram_tensor("out_b_eff", [P, 1], F32, kind="ExternalInput")
    out_d = nc.dram_tensor("out", [P, N], F32, kind="ExternalOutput")

    EXP = mybir.ActivationFunctionType.Exp
    MULT = mybir.AluOpType.mult
    ADD = mybir.AluOpType.add

    with tile.TileContext(nc) as tc:
        const = tc.alloc_tile_pool(name="const", bufs=1)
        big = tc.alloc_tile_pool(name="big", bufs=1)
        probs_p = tc.alloc_tile_pool(name="probs", bufs=4)
        ysb_p = tc.alloc_tile_pool(name="ysb", bufs=2)
        rs_p = tc.alloc_tile_pool(name="rs", bufs=2)
        inv_p = tc.alloc_tile_pool(name="inv", bufs=2)
        osb_p = tc.alloc_tile_pool(name="osb", bufs=3)
        ssb_p = tc.alloc_tile_pool(name="ssb", bufs=2)

        # PSUM budget (8 banks): aux 2x1 + scores 2x2 + y 1x2 = 8
        aux_ps = tc.alloc_tile_pool(name="aux_ps", bufs=2, space="PSUM")
        sc_ps = tc.alloc_tile_pool(name="sc_ps", bufs=2, space="PSUM")
        y_ps = tc.alloc_tile_pool(name="y_ps", bufs=1, space="PSUM")

        # ---- warm up ACT exp table immediately (no data deps) ----
        ones_col = const.tile([P, 1], F32, tag='ones_col')
        nc.vector.memset(ones_col[:], 1.0)
        warm = const.tile([1, 1], F32, tag='warm')
        nc.scalar.activation(warm[:], ones_col[0:1, :], EXP)

        # ---- constants / weights ----
        ulhs = const.tile([P, P], F32, tag='ulhs')
        nc.sync.dma_start(ulhs[:], ulhs_d.ap())
        v_sb = const.tile([P, 1], F32, tag='v')
        nc.gpsimd.dma_start(v_sb[:], v_d.ap())
        gwt = const.tile([P, INTER], F32, tag='gwt')
        nc.gpsimd.dma_start(gwt[:], gwt_d.ap())
        owt = const.tile([INTER, P], F32, tag='owt')
        nc.gpsimd.dma_start(owt[:], owt_d.ap())
        obe = const.tile([P, 1], F32, tag='obe')
        nc.gpsimd.dma_start(obe[:], obe_d.ap())
        ulhs_r = const.tile([P, P], BF16, tag='ulhsr')
        nc.vector.tensor_copy(ulhs_r[:], ulhs[:])
        gwt_r = const.tile([P, INTER], BF16, tag='gwtr')
        nc.vector.tensor_copy(gwt_r[:], gwt[:])
        owt_r = const.tile([INTER, P], BF16, tag='owtr')
        nc.vector.tensor_copy(owt_r[:], owt[:])
        ones_row = const.tile([1, P], F32, tag='ones_row')
        nc.vector.memset(ones_row[:], 1.0)
        e_col = const.tile([P, 1], F32, tag='e_col')
        nc.vector.memset(e_col[:], float(np.exp(1.0)))

        x_sb = big.tile([P, N], F32, tag='x')
        x_r = big.tile([P, N], BF16, tag='xr')
        u_sb = big.tile([P, N], BF16, tag='u')
        xpb = big.tile([P, N], F32, tag='xpb')
        g_aug = big.tile([P, MBLK * (INTER + 1)], BF16, tag='gaug')

        # ---- prologue: bf16 x lands first in small pieces, f32 x later ----
        xbf_slices = [(0, 512), (512, 512), (1024, 1024), (2048, 1024), (3072, 1024)]
        for (o, w) in xbf_slices:
            nc.sync.dma_start(x_r[:, o:o + w], xbf_d.ap()[:, o:o + w])
        for c in range(N // NCH):
            sl = slice(c * NCH, (c + 1) * NCH)
            nc.sync.dma_start(x_sb[:, sl], x_d.ap()[:, sl])
        for (o, w) in xbf_slices:
            for h in range(w // NSUB):
                sl = slice(o + h * NSUB, o + (h + 1) * NSUB)
                u_pt = aux_ps.tile([P, NSUB], F32, tag="aux")
                nc.tensor.matmul(u_pt[:], ulhs_r[:], x_r[:, sl],
                                 start=True, stop=True)
                nc.vector.tensor_scalar_add(u_sb[:, sl], u_pt[:], v_sb[:])
            for j in range(o // P, (o + w) // P):
                g_pt = aux_ps.tile([P, INTER], F32, tag="aux")
                nc.tensor.matmul(g_pt[:], x_r[:, j * P:(j + 1) * P], gwt_r[:],
                                 start=True, stop=True)
                nc.vector.tensor_copy(g_aug[:, j * 65:j * 65 + INTER], g_pt[:])
        for c in range(N // NCH):
            csl = slice(c * NCH, (c + 1) * NCH)
            nc.vector.tensor_scalar_add(xpb[:, csl], x_sb[:, csl], obe[:])
        g_ones_view = g_aug.rearrange("p (j t) -> p j t", t=INTER + 1)[:, :, INTER:INTER + 1]
        nc.vector.tensor_copy(g_ones_view, ones_col[:].to_broadcast([P, MBLK, 1]))

        # ---- main loop over n-chunks ----
        NC_CHUNKS = N // NCH
        GP_BLOCKS = ()              # gpsimd pow-exp offload: measured far too slow
        pending_out = []

        def emit_out_phase(c, y_t):
            n0 = c * NCH
            last = (c == NC_CHUNKS - 1)
            y_sb = ysb_p.tile([INTER, NCH], BF16, name="y_sb")
            nc.vector.tensor_copy(y_sb[:], y_t[0:INTER, :])
            rs_sb = rs_p.tile([1, NCH], F32, name="rs_sb")
            if last:
                nc.scalar.copy(rs_sb[:], y_t[INTER:INTER + 1, :])
            else:
                nc.vector.tensor_copy(rs_sb[:], y_t[INTER:INTER + 1, :])
            inv_sb = inv_p.tile([P, NCH], F32, tag='inv', name="inv_sb")
            if last:
                for h in range(NCH // NSUB):
                    sl = slice(h * NSUB, (h + 1) * NSUB)
                    bc_t = aux_ps.tile([P, NSUB], F32, tag="aux", name="bc_t")
                    nc.tensor.matmul(bc_t[:], ones_row[:],
                                     rs_sb[:, sl], start=True, stop=True)
                    nc.vector.reciprocal_approx_fast(out=inv_sb[:, sl], in_=bc_t[:])
            else:
                rs_bc = inv_p.tile([P, NCH], F32, tag='rsbc', name="rs_bc")
                nc.gpsimd.partition_broadcast(rs_bc[:], rs_sb[:])
                nc.vector.reciprocal_approx_fast(out=inv_sb[:], in_=rs_bc[:])
            for h in range(NCH // NSUB):
                sl = slice(h * NSUB, (h + 1) * NSUB)
                ot = aux_ps.tile([P, NSUB], F32, tag="aux", name="ot")
                nc.tensor.matmul(ot[:], owt_r[:], y_sb[:, sl],
                                 start=True, stop=True)
                t_sb = osb_p.tile([P, NSUB], F32, name="t_sb")
                nc.vector.tensor_tensor(t_sb[:], ot[:], inv_sb[:, sl], op=MULT)
                nc.vector.tensor_tensor(
                    t_sb[:], t_sb[:],
                    xpb[:, n0 + h * NSUB:n0 + (h + 1) * NSUB], op=ADD)
                nc.sync.dma_start(out_d.ap()[:, n0 + h * NSUB:n0 + (h + 1) * NSUB],
                                  t_sb[:])

        for c in range(NC_CHUNKS):
            n0 = c * NCH
            gp_blocks = GP_BLOCKS if c < NC_CHUNKS - 1 else ()
            act_blocks = [j for j in range(MBLK) if j not in gp_blocks]
            y_t = y_ps.tile([INTER + 1, NCH], F32)

            def scores_mm(j, s_t):
                for h in range(NCH // NSUB):
                    nc.tensor.matmul(
                        s_t[:, h * NSUB:(h + 1) * NSUB],
                        x_r[:, j * P:(j + 1) * P],
                        u_sb[:, n0 + h * NSUB:n0 + (h + 1) * NSUB],
                        start=True, stop=True)

            def y_mm(j, p_t, start, stop):
                for h in range(NCH // NSUB):
                    nc.tensor.matmul(
                        y_t[:, h * NSUB:(h + 1) * NSUB],
                        g_aug[:, j * 65:(j + 1) * 65],
                        p_t[:, h * NSUB:(h + 1) * NSUB],
                        start=start and h < 2, stop=stop)

            # first block's scores+exp go ahead of the previous chunk's
            # out-phase so the ACT stream doesn't stall at the boundary
            j0 = act_blocks[0]
            s_t = sc_ps.tile([P, NCH], F32, name="s_t")
            scores_mm(j0, s_t)
            p0 = probs_p.tile([P, NCH], BF16, name="p_t")
            nc.scalar.activation(p0[:], s_t[:], EXP)
            if pending_out:
                emit_out_phase(*pending_out.pop(0))
            y_mm(j0, p0, start=True, stop=False)

            for k, j in enumerate(act_blocks[1:]):
                s_t = sc_ps.tile([P, NCH], F32, name="s_t")
                scores_mm(j, s_t)
                p_t = probs_p.tile([P, NCH], BF16, name="p_t")
                nc.scalar.activation(p_t[:], s_t[:], EXP)
                y_mm(j, p_t, start=False, stop=(k == len(act_blocks) - 2))

            pending_out.append((c, y_t))

        while pending_out:
            emit_out_phase(*pending_out.pop(0))

        for p in (y_ps, sc_ps, aux_ps, ssb_p,
                  osb_p, inv_p, rs_p, ysb_p, probs_p, big, const):
            p.release()

    nc.compile()
    return nc


def _prep_inputs(x, theta_w, theta_b, phi_w, phi_b, g_w, g_b, out_w, out_b):
    f = np.float32
    x = np.asarray(x, f)
    theta_w = np.asarray(theta_w, f)
    theta_b = np.asarray(theta_b, f)
    phi_w = np.asarray(phi_w, f)
    phi_b = np.asarray(phi_b, f)
    g_w = np.asarray(g_w, f)
    g_b = np.asarray(g_b, f)
    out_w = np.asarray(out_w, f)
    out_b = np.asarray(out_b, f)

    u_lhsT = np.ascontiguousarray(theta_w.T @ phi_w)          # [c2, c1] = B^T
    v = np.ascontiguousarray((phi_w.T @ theta_b)[:, None])    # [128, 1]
    g_wT = np.ascontiguousarray(g_w.T)                        # [128, 64]
    out_wT = np.ascontiguousarray(out_w.T)                    # [64, 128]
    out_b_eff = np.ascontiguousarray((out_w @ g_b + out_b)[:, None])

    in_maps = []
    for b in range(B):
        in_maps.append({
            "x": np.ascontiguousarray(x[b].reshape(P, N)),
            "x_bf": np.ascontiguousarray(
                x[b].reshape(P, N).astype(ml_dtypes.bfloat16)),
            "u_lhsT": u_lhsT,
            "v": v,
            "g_wT": g_wT,
            "out_wT": out_wT,
            "out_b_eff": out_b_eff,
        })
    return in_maps


def run_on_device(inputs, trace=False, trace_cores=None):
    if "nc" not in _cached:
        _cached["nc"] = build_nc()
    nc = _cached["nc"]
    in_maps = _prep_inputs(**inputs)
    res = bass_utils.run_bass_kernel_spmd(
        nc, in_maps, core_ids=list(range(B)), trace=trace,
        trace_cores=trace_cores)
    out = np.stack([res.results[b]["out"] for b in range(B)], axis=0)
    return out.reshape(B, C, H, W).astype(np.float32), res


def kernel(**inputs):
    out, _ = run_on_device(inputs, trace=False)
    return out
